# revision 1
# baseline (speedup 1.0000x reference)
"""Trainium2 Bass kernel for nn_Conditioning (embedding lookup + concat).

Reference computation:
    gc = W.T[ids] + b          # (B, T, 64) gather from a tiny 128x64 table
    out = concat(lc, gc, -1)   # (B, T, 128)

Shapes: lc (16, 32768, 64) f32, ids (16, 32768) int64, W (64, 128) f32,
b (64,) f32 -> out (16, 32768, 128) f32.

Sharding: data-parallel over batch — 2 batches (65536 tokens) per core on
8 cores; W and b replicated.

Device algorithm (per core), memory-roofline oriented (~48 MB HBM traffic
= ~134 us at 358 GB/s/core):
  * One-time: build WTb = W.T + b in SBUF (bias broadcast across partitions
    via GpSimd partition_broadcast), then split into a packed bf16 table
    wtbx = [bf16(WTb) | bf16(WTb - bf16(WTb))] (hi|lo halves) so the gather
    is exact to ~2^-16 relative after the hi+lo re-add; iota column
    (partition index, f32) for one-hot building.
  * Per macro-tile of 128*Q tokens (token t = Q*p + q <-> partition p,
    slot q; Q=32 steady state, with a short Q=8 ramp-up prologue so the
    first stores issue early and the DMA engines never idle):
      - DMA ids row (1, 128*Q) bf16 (ScalarE HWDGE);
        GpSimd partition_broadcast -> (128, 128*Q) bf16.
      - VectorE is_equal(ids_bcast, iota) -> one-hot (speaker, token) bf16.
      - Q matmuls (one-hot (128,128) stationary, packed wtbx (128,128)
        moving) -> PSUM (token-slot, [hi64|lo64]) f32, 8 slots per PSUM
        tile (2 banks, 4 bufs).
      - hi half copied into the gc columns of the assembled out tile
        (ScalarE/VectorE alternating); VectorE adds the lo PSUM half in
        place (exact f32 re-add, one PSUM operand per op).
      - DMA lc into a contiguous staging tile (Sync HWDGE); ScalarE
        copies it into the interleaved lc columns of the out tile.
      - One fully contiguous store per macro (Sync HWDGE, 2 MB steady
        state).
"""

import sys

for _p in ("/opt/trn_rl_repo",):
    if _p not in sys.path:
        sys.path.insert(0, _p)

from contextlib import ExitStack

import ml_dtypes
import numpy as np

import concourse.bass as bass  # noqa: F401
import concourse.tile as tile
from concourse import bacc, mybir
from concourse.bass_utils import run_bass_kernel_spmd

N_CORES = 8
B, T, I = 16, 32768, 64
N_SPK, N_EMBED = 128, 64
P = 128  # partitions
TOK_PER_CORE = B * T // N_CORES  # 65536
# (tokens-per-partition Q, macro count): short ramp-up then 4096-token macros
SCHEDULE = ((8, 4), (32, 15))
CHUNK = 8  # psum rotation granularity (8 slots = 2 banks, 4 bufs)

F32 = mybir.dt.float32
BF16 = mybir.dt.bfloat16

assert sum(P * q * c for q, c in SCHEDULE) == TOK_PER_CORE


def _macro_list(schedule):
    tok0, out = 0, []
    for q, cnt in schedule:
        for _ in range(cnt):
            out.append((tok0, q))
            tok0 += P * q
    return out, tok0


def build_bass(schedule=SCHEDULE):
    macros, tok = _macro_list(schedule)
    max_q = max(q for _, q in macros)

    nc = bacc.Bacc("TRN2", target_bir_lowering=False, debug=False)
    lc = nc.dram_tensor("lc", (tok, I), F32, kind="ExternalInput").ap()
    ids = nc.dram_tensor("ids", (tok,), BF16, kind="ExternalInput").ap()
    wt = nc.dram_tensor("wt", (N_SPK, N_EMBED), F32, kind="ExternalInput").ap()
    bi = nc.dram_tensor("bias", (1, N_EMBED), F32, kind="ExternalInput").ap()
    out = nc.dram_tensor("out", (tok, I + N_EMBED), F32, kind="ExternalOutput").ap()

    with tile.TileContext(nc) as tc, ExitStack() as ctx:
        const = ctx.enter_context(tc.tile_pool(name="const", bufs=1))
        ids_pool = ctx.enter_context(tc.tile_pool(name="idsrow", bufs=3))
        bc_pool = ctx.enter_context(tc.tile_pool(name="idsbc", bufs=2))
        oh_pool = ctx.enter_context(tc.tile_pool(name="onehot", bufs=2))
        lc_pool = ctx.enter_context(tc.tile_pool(name="lct", bufs=5))
        out_pool = ctx.enter_context(tc.tile_pool(name="outt", bufs=3))
        pgc_pool = ctx.enter_context(tc.tile_pool(name="pgc", bufs=4, space="PSUM"))

        # ---- one-time constants ----
        wt_sb = const.tile([N_SPK, N_EMBED], F32)
        nc.sync.dma_start(out=wt_sb[:], in_=wt[:])
        b_row = const.tile([1, N_EMBED], F32)
        nc.sync.dma_start(out=b_row[:], in_=bi[:])
        b_bc = const.tile([N_SPK, N_EMBED], F32)
        nc.gpsimd.partition_broadcast(b_bc[:], b_row[:])
        wtb = const.tile([N_SPK, N_EMBED], F32)
        nc.vector.tensor_tensor(
            out=wtb[:], in0=wt_sb[:], in1=b_bc[:], op=mybir.AluOpType.add
        )
        # packed bf16 table: [hi | lo]
        wtbx = const.tile([N_SPK, 2 * N_EMBED], BF16)
        nc.vector.tensor_copy(out=wtbx[:, 0:N_EMBED], in_=wtb[:])
        hi_f32 = const.tile([N_SPK, N_EMBED], F32)
        nc.vector.tensor_copy(out=hi_f32[:], in_=wtbx[:, 0:N_EMBED])
        nc.vector.tensor_tensor(
            out=wtbx[:, N_EMBED : 2 * N_EMBED],
            in0=wtb[:],
            in1=hi_f32[:],
            op=mybir.AluOpType.subtract,
        )
        iota_i = const.tile([P, 1], mybir.dt.int32)
        nc.gpsimd.iota(iota_i[:], pattern=[[0, 1]], base=0, channel_multiplier=1)
        iota_f = const.tile([P, 1], F32)
        nc.vector.tensor_copy(out=iota_f[:], in_=iota_i[:])

        # ---- main loop ----
        for tok0, q in macros:
            macro = P * q
            lc_re = lc[tok0 : tok0 + macro, :].rearrange("(p q) d -> p (q d)", p=P, q=q)
            out_re = out[tok0 : tok0 + macro, :].rearrange(
                "(p q) d -> p (q d)", p=P, q=q
            )
            ids_re = ids[tok0 : tok0 + macro].rearrange("(o m) -> o m", o=1)

            ids_row = ids_pool.tile([1, macro], BF16, tag="ids_row")
            nc.scalar.dma_start(out=ids_row[:], in_=ids_re)
            ids_bc = bc_pool.tile([P, macro], BF16, tag="ids_bc")
            nc.gpsimd.partition_broadcast(ids_bc[:], ids_row[:])
            onehot = oh_pool.tile([P, macro], BF16, tag="onehot")
            nc.vector.tensor_scalar(
                out=onehot[:],
                in0=ids_bc[:],
                scalar1=iota_f[:],
                scalar2=None,
                op0=mybir.AluOpType.is_equal,
            )

            lc_t = lc_pool.tile([P, q * I], F32, tag="lc_t")
            nc.sync.dma_start(out=lc_t[:], in_=lc_re)

            out_t = out_pool.tile([P, q, I + N_EMBED], F32, tag="out_t")
            chunk = min(CHUNK, q)
            for h in range(q // chunk):
                sl = slice(h * chunk, (h + 1) * chunk)
                psum_gc = pgc_pool.tile([P, chunk, 2 * N_EMBED], F32, tag="psum_gc")
                for jj in range(chunk):
                    j = h * chunk + jj
                    nc.tensor.matmul(
                        psum_gc[:, jj, :],
                        lhsT=onehot[:, j * P : (j + 1) * P],
                        rhs=wtbx[:],
                        start=True,
                        stop=True,
                    )
                # hi half -> out tile (ACT and DVE alternate chunks), then
                # the lo half is added in place — exact f32 re-add with a
                # single PSUM operand per DVE op
                if h % 2 == 0:
                    nc.scalar.copy(
                        out_t[:, sl, I : I + N_EMBED], psum_gc[:, :, 0:N_EMBED]
                    )
                else:
                    nc.vector.tensor_copy(
                        out=out_t[:, sl, I : I + N_EMBED],
                        in_=psum_gc[:, :, 0:N_EMBED],
                    )
                nc.vector.tensor_tensor(
                    out=out_t[:, sl, I : I + N_EMBED],
                    in0=psum_gc[:, :, N_EMBED : 2 * N_EMBED],
                    in1=out_t[:, sl, I : I + N_EMBED],
                    op=mybir.AluOpType.add,
                )
            # interleave lc into the out tile
            nc.scalar.copy(out_t[:, :, 0:I], lc_t[:])
            nc.sync.dma_start(out=out_re, in_=out_t[:])

    nc.compile()
    return nc


_NC_CACHE: dict = {}


def _get_nc(schedule=SCHEDULE):
    if schedule not in _NC_CACHE:
        _NC_CACHE[schedule] = build_bass(schedule)
    return _NC_CACHE[schedule]


def prep_ids(ids_shard_flat, schedule=SCHEDULE):
    """bf16-encode and slot-group a per-core flat ids shard.

    Within each macro of 128*q tokens, token t = q*p + s must appear at
    column s*128 + p so that matmul group s's one-hot columns line up with
    PSUM slot p (pure layout permutation; values unchanged).
    """
    a = np.asarray(ids_shard_flat).astype(np.float32).astype(ml_dtypes.bfloat16)
    macros, tok = _macro_list(schedule)
    assert a.shape == (tok,)
    parts = []
    for tok0, q in macros:
        parts.append(a[tok0 : tok0 + P * q].reshape(P, q).T.reshape(-1))
    return np.ascontiguousarray(np.concatenate(parts))


def make_in_maps(lc, ids, W, b):
    """Shard full inputs into per-core input maps for the bass kernel."""
    lc_flat = np.ascontiguousarray(np.asarray(lc, dtype=np.float32)).reshape(B * T, I)
    ids_flat = np.asarray(ids).reshape(B * T)
    wt = np.ascontiguousarray(np.asarray(W, dtype=np.float32).T)  # (128, 64)
    bi = np.asarray(b, dtype=np.float32).reshape(1, N_EMBED)
    in_maps = []
    for c in range(N_CORES):
        s = slice(c * TOK_PER_CORE, (c + 1) * TOK_PER_CORE)
        in_maps.append(
            {
                "lc": lc_flat[s],
                "ids": prep_ids(ids_flat[s]),
                "wt": wt,
                "bias": bi,
            }
        )
    return in_maps


_SHARDED_CACHE: dict = {}


def _get_sharded(nc):
    """Build (once) and cache the jitted SPMD executable for `nc`.

    Mirrors the multi-core branch of bass2jax.run_bass_via_pjrt, but keeps
    the jitted function across kernel() invocations — the stock path builds
    a fresh closure per call, which forces a full jax re-trace/compile each
    time (~7-9 s of repeat-call wall time).
    """
    if "entry" in _SHARDED_CACHE:
        return _SHARDED_CACHE["entry"]

    import jax
    from jax.experimental.shard_map import shard_map
    from jax.sharding import Mesh, PartitionSpec

    from concourse import bass2jax, mybir as _mybir

    bass2jax.install_neuronx_cc_hook()
    assert nc.dbg_addr is None
    partition_name = nc.partition_id_tensor.name if nc.partition_id_tensor else None

    in_names, out_names, out_avals = [], [], []
    for alloc in nc.m.functions[0].allocations:
        if not isinstance(alloc, _mybir.MemoryLocationSet):
            continue
        name = alloc.memorylocations[0].name
        if alloc.kind == "ExternalInput":
            if name != partition_name:
                in_names.append(name)
        elif alloc.kind == "ExternalOutput":
            shape = tuple(alloc.tensor_shape)
            out_avals.append(jax.core.ShapedArray(shape, _mybir.dt.np(alloc.dtype)))
            out_names.append(name)
    n_params, n_outs = len(in_names), len(out_names)
    all_names = in_names + out_names
    if partition_name is not None:
        all_names = all_names + [partition_name]
    donate = tuple(range(n_params, n_params + n_outs))

    def _body(*args):
        operands = list(args)
        if partition_name is not None:
            operands.append(bass2jax.partition_id_tensor())
        outs = bass2jax._bass_exec_p.bind(
            *operands,
            out_avals=tuple(out_avals),
            in_names=tuple(all_names),
            out_names=tuple(out_names),
            lowering_input_output_aliases=(),
            sim_require_finite=True,
            sim_require_nnan=True,
            nc=nc,
        )
        return tuple(outs)

    devices = jax.devices()[:N_CORES]
    mesh = Mesh(np.asarray(devices), ("core",))
    in_specs = (PartitionSpec("core"),) * (n_params + n_outs)
    out_specs = (PartitionSpec("core"),) * n_outs
    sharded = jax.jit(
        shard_map(
            _body, mesh=mesh, in_specs=in_specs, out_specs=out_specs, check_rep=False
        ),
        donate_argnums=donate,
        keep_unused=True,
    )
    entry = (sharded, in_names, out_names, out_avals)
    _SHARDED_CACHE["entry"] = entry
    return entry


def make_concat_inputs(lc, ids, W, b):
    """Globally concatenated (axis 0) per-core inputs for the cached SPMD
    path — avoids the per-core slice -> re-concat round-trip copies."""
    lc_flat = np.ascontiguousarray(np.asarray(lc, dtype=np.float32)).reshape(B * T, I)
    ids_flat = np.asarray(ids).reshape(B * T)
    ids_all = np.concatenate(
        [
            prep_ids(ids_flat[c * TOK_PER_CORE : (c + 1) * TOK_PER_CORE])
            for c in range(N_CORES)
        ]
    )
    wt = np.ascontiguousarray(np.asarray(W, dtype=np.float32).T)
    bi = np.asarray(b, dtype=np.float32).reshape(1, N_EMBED)
    return {
        "lc": lc_flat,
        "ids": ids_all,
        "wt": np.tile(wt, (N_CORES, 1)),
        "bias": np.tile(bi, (N_CORES, 1)),
    }


def _run_spmd_cached(nc, concat_inputs):
    """Returns the full concatenated output (B*T, 128)."""
    sharded, in_names, out_names, out_avals = _get_sharded(nc)
    concat_in = [concat_inputs[name] for name in in_names]
    concat_zeros = [
        np.zeros((N_CORES * a.shape[0], *a.shape[1:]), a.dtype) for a in out_avals
    ]
    out_arrs = sharded(*concat_in, *concat_zeros)
    i = out_names.index("out")
    return np.asarray(out_arrs[i]).reshape(B * T, I + N_EMBED)


def run(lc, ids, W, b, trace: bool = False):
    """Run on 8 NeuronCores; returns (full_output, BassKernelResults)."""
    nc = _get_nc()
    res = None
    try:
        out_flat = _run_spmd_cached(nc, make_concat_inputs(lc, ids, W, b))
    except Exception as e:  # noqa: BLE001 — fall back to the stock path
        print(f"kernel: cached SPMD path failed ({e!r}); using run_bass_kernel_spmd")
        in_maps = make_in_maps(lc, ids, W, b)
        res = run_bass_kernel_spmd(nc, in_maps, list(range(N_CORES)), trace=trace)
        out_flat = np.concatenate(
            [res.results[c]["out"] for c in range(N_CORES)], axis=0
        )
    out = out_flat.reshape(B, T, I + N_EMBED)
    return np.ascontiguousarray(out, dtype=np.float32), res


def kernel(lc, ids, W, b):
    out, _ = run(lc, ids, W, b)
    return out


if __name__ == "__main__":
    rng = np.random.default_rng(0)
    lc = rng.standard_normal((B, T, I), dtype=np.float32)
    ids = rng.integers(0, N_SPK, size=(B, T), dtype=np.int64)
    W = rng.standard_normal((N_EMBED, N_SPK), dtype=np.float32)
    b = rng.standard_normal((N_EMBED,), dtype=np.float32)
    out = kernel(lc=lc, ids=ids, W=W, b=b)
    exp = np.concatenate((lc, W.T[ids] + b), axis=2)
    err = np.max(np.abs(out - exp)) / np.max(np.abs(exp))
    print("max abs rel-to-scale err:", err)



# revision 7
# speedup vs baseline: 1.0549x; 1.0549x over previous
"""Trainium2 Bass kernel for nn_Conditioning (embedding lookup + concat).

Reference computation:
    gc = W.T[ids] + b          # (B, T, 64) gather from a tiny 128x64 table
    out = concat(lc, gc, -1)   # (B, T, 128)

Shapes: lc (16, 32768, 64) f32, ids (16, 32768) int64, W (64, 128) f32,
b (64,) f32 -> out (16, 32768, 128) f32.

Sharding: data-parallel over batch — 2 batches (65536 tokens) per core on
8 cores; the packed table and iota column replicated.

Device algorithm (per core), memory-roofline oriented (~50.5 MB DMA traffic
= ~140.3 us at 360 GB/s/core — loads + stores serialize on the DMA bus):
  * Host precomputes the packed bf16 gather table
    wtbx = [bf16(W.T + b) | bf16((W.T + b) - f32(bf16(W.T + b)))]  (hi|lo)
    and a (128, 1) f32 iota column, so the device does no constant setup.
  * ids are host-encoded to bf16 and slot-permuted per macro-tile.
  * Per macro-tile of 128*q tokens (token t = q*p + s <-> partition p,
    slot s; q=32 steady state with a short ramp-up):
      - ids row (1, 128q) bf16 DMA (Act HWDGE queue); GpSimd
        partition_broadcast of the row VIEWED AS int32 pairs (halves the
        modeled Q7 time, byte-identical result) -> (128, 128q) bf16.
      - DVE is_equal(ids_bcast, iota) -> one-hot (speaker, token) bf16,
        all-SBUF 2-byte operands (4x DVE mode).
      - Per 8-slot chunk: 8 matmul PAIRS accumulate hi+lo halves of the
        table into one (128, 8, 64) f32 PSUM bank (exact f32 re-add done
        by PE accumulation, same modeled PE cost as one 128-wide matmul);
        then a single copy PSUM -> out-tile gc columns (Act/DVE alternate).
      - lc DMA into staging (Act queue); Act copies it into the out tile's
        lc columns one half-macro at a time.
      - TWO contiguous stores per macro (half-macro each, SP queue) so
        stores start earlier and the final tail is short. Loads and stores
        live on different HWDGE queues so a store's semaphore wait never
        blocks the next load's issue; loads run 3 macros ahead.
"""

import sys

for _p in ("/opt/trn_rl_repo",):
    if _p not in sys.path:
        sys.path.insert(0, _p)

from contextlib import ExitStack

import ml_dtypes
import numpy as np

import concourse.bass as bass  # noqa: F401
import concourse.tile as tile
from concourse import bacc, mybir
from concourse.bass_utils import run_bass_kernel_spmd

N_CORES = 8
B, T, I = 16, 32768, 64
N_SPK, N_EMBED = 128, 64
P = 128  # partitions
TOK_PER_CORE = B * T // N_CORES  # 65536
# (tokens-per-partition q, macro count): short ramp-up then 4096-token macros
SCHEDULE = ((32, 16),)
CHUNK = 8  # psum tile = (128, CHUNK, 64) f32 = one 2 KB PSUM bank
LOOKAHEAD = 3  # macros of load lookahead on the Act queue

F32 = mybir.dt.float32
BF16 = mybir.dt.bfloat16
I32 = mybir.dt.int32

assert sum(P * q * c for q, c in SCHEDULE) == TOK_PER_CORE


def _macro_list(schedule):
    tok0, out = 0, []
    for q, cnt in schedule:
        for _ in range(cnt):
            out.append((tok0, q))
            tok0 += P * q
    return out, tok0


def build_bass(schedule=SCHEDULE):
    macros, tok = _macro_list(schedule)
    n_macros = len(macros)

    nc = bacc.Bacc("TRN2", target_bir_lowering=False, debug=False)
    lc = nc.dram_tensor("lc", (tok, I), F32, kind="ExternalInput").ap()
    ids = nc.dram_tensor("ids", (tok,), BF16, kind="ExternalInput").ap()
    wtbx_d = nc.dram_tensor("wtbx", (N_SPK, 2 * N_EMBED), BF16, kind="ExternalInput").ap()
    iota_d = nc.dram_tensor("iota", (P, 1), F32, kind="ExternalInput").ap()
    out = nc.dram_tensor("out", (tok, I + N_EMBED), F32, kind="ExternalOutput").ap()

    with tile.TileContext(nc) as tc, ExitStack() as ctx:
        const = ctx.enter_context(tc.tile_pool(name="const", bufs=1))
        ids_pool = ctx.enter_context(tc.tile_pool(name="idsrow", bufs=LOOKAHEAD + 2))
        bc_pool = ctx.enter_context(tc.tile_pool(name="idsbc", bufs=2))
        oh_pool = ctx.enter_context(tc.tile_pool(name="onehot", bufs=2))
        lc_pool = ctx.enter_context(tc.tile_pool(name="lct", bufs=LOOKAHEAD + 2))
        out_pool = ctx.enter_context(tc.tile_pool(name="outt", bufs=3))
        pgc_pool = ctx.enter_context(tc.tile_pool(name="pgc", bufs=6, space="PSUM"))

        # ---- load issue (Act queue), LOOKAHEAD macros ahead of compute ----
        t_ids, t_lc = {}, {}

        def load(k):
            tok0, q = macros[k]
            macro = P * q
            t_ids[k] = ids_pool.tile([1, macro], BF16, tag="ids_row", name=f"ids_row_{k}")
            nc.scalar.dma_start(
                out=t_ids[k][:], in_=ids[tok0 : tok0 + macro].rearrange("(o m) -> o m", o=1)
            )
            t_lc[k] = lc_pool.tile([P, q * I], F32, tag="lc_t", name=f"lc_t_{k}")
            # macro 0's lc rides the (otherwise idle) SP queue: it becomes the
            # very first HWDGE issue, so the big first transfer starts ~1.4 us
            # in while ids0 issues in parallel on Act
            eng = nc.sync if k == 0 else nc.scalar
            eng.dma_start(
                out=t_lc[k][:],
                in_=lc[tok0 : tok0 + macro, :].rearrange("(p q) d -> p (q d)", p=P, q=q),
            )

        def compute(k):
            tok0, q = macros[k]
            macro = P * q
            ids_row = t_ids.pop(k)
            lc_t = t_lc.pop(k)

            ids_bc = bc_pool.tile([P, macro], BF16, tag="ids_bc")
            nc.gpsimd.partition_broadcast(
                ids_bc[:].bitcast(I32), ids_row[:].bitcast(I32)
            )
            onehot = oh_pool.tile([P, macro], BF16, tag="onehot")
            nc.vector.tensor_scalar(
                out=onehot[:],
                in0=ids_bc[:],
                scalar1=iota_f[:],
                scalar2=None,
                op0=mybir.AluOpType.is_equal,
            )

            out_t = out_pool.tile([P, q, I + N_EMBED], F32, tag="out_t")
            out_re = out[tok0 : tok0 + macro, :].rearrange(
                "(p q) d -> p q d", p=P, q=q
            )
            chunk = min(CHUNK, q)
            n_chunks = q // chunk
            halves = 2 if n_chunks >= 2 else 1
            per_half = n_chunks // halves
            q2 = q // halves
            for s in range(halves):
                for hh in range(per_half):
                    h = s * per_half + hh
                    sl = slice(h * chunk, (h + 1) * chunk)
                    pg = pgc_pool.tile([P, chunk, N_EMBED], F32, tag="pg")
                    for jj in range(chunk):
                        j = h * chunk + jj
                        nc.tensor.matmul(
                            pg[:, jj, :],
                            lhsT=onehot[:, j * P : (j + 1) * P],
                            rhs=wtbx[:, 0:N_EMBED],
                            start=True,
                            stop=False,
                        )
                        nc.tensor.matmul(
                            pg[:, jj, :],
                            lhsT=onehot[:, j * P : (j + 1) * P],
                            rhs=wtbx[:, N_EMBED : 2 * N_EMBED],
                            start=False,
                            stop=True,
                        )
                    # single exact copy: PSUM already holds hi+lo re-added
                    if h % 2 == 0:
                        nc.scalar.copy(out_t[:, sl, I : I + N_EMBED], pg[:])
                    else:
                        nc.vector.tensor_copy(
                            out=out_t[:, sl, I : I + N_EMBED], in_=pg[:]
                        )
                shalf = slice(s * q2, (s + 1) * q2)
                nc.scalar.copy(
                    out_t[:, shalf, 0:I], lc_t[:, s * q2 * I : (s + 1) * q2 * I]
                )
                nc.sync.dma_start(out=out_re[:, shalf, :], in_=out_t[:, shalf, :])

        # first macro's loads are the very first HWDGE issues (ids0 then lc0);
        # the two tiny constants ride the Pool SWDGE path so they never take
        # an HWDGE slot in the critical head window
        load(0)
        iota_f = const.tile([P, 1], F32)
        nc.gpsimd.dma_start(out=iota_f[:], in_=iota_d[:])
        wtbx = const.tile([N_SPK, 2 * N_EMBED], BF16)
        nc.gpsimd.dma_start(out=wtbx[:], in_=wtbx_d[:])
        for k in range(1, min(LOOKAHEAD, n_macros)):
            load(k)
        for k in range(n_macros):
            if k + LOOKAHEAD < n_macros:
                load(k + LOOKAHEAD)
            compute(k)

    nc.compile()
    return nc


_NC_CACHE: dict = {}


def _get_nc(schedule=SCHEDULE):
    if schedule not in _NC_CACHE:
        _NC_CACHE[schedule] = build_bass(schedule)
    return _NC_CACHE[schedule]


def prep_ids(ids_shard_flat, schedule=SCHEDULE):
    """bf16-encode and slot-group a per-core flat ids shard.

    Within each macro of 128*q tokens, token t = q*p + s must appear at
    column s*128 + p so that matmul group s's one-hot columns line up with
    PSUM slot s (pure layout permutation; values unchanged).
    """
    a = np.asarray(ids_shard_flat).astype(np.float32).astype(ml_dtypes.bfloat16)
    macros, tok = _macro_list(schedule)
    assert a.shape == (tok,)
    parts = []
    for tok0, q in macros:
        parts.append(a[tok0 : tok0 + P * q].reshape(P, q).T.reshape(-1))
    return np.ascontiguousarray(np.concatenate(parts))


def prep_wtbx(W, b):
    """Packed bf16 hi|lo gather table: rows = speaker, cols = [hi64 | lo64]."""
    wtb = np.asarray(W, dtype=np.float32).T + np.asarray(b, dtype=np.float32)[None, :]
    hi = wtb.astype(ml_dtypes.bfloat16)
    lo = (wtb - hi.astype(np.float32)).astype(ml_dtypes.bfloat16)
    return np.ascontiguousarray(np.concatenate([hi, lo], axis=1))


_IOTA = np.arange(P, dtype=np.float32).reshape(P, 1)


def make_in_maps(lc, ids, W, b):
    """Shard full inputs into per-core input maps for the bass kernel."""
    lc_flat = np.ascontiguousarray(np.asarray(lc, dtype=np.float32)).reshape(B * T, I)
    ids_flat = np.asarray(ids).reshape(B * T)
    wtbx = prep_wtbx(W, b)
    in_maps = []
    for c in range(N_CORES):
        s = slice(c * TOK_PER_CORE, (c + 1) * TOK_PER_CORE)
        in_maps.append(
            {
                "lc": lc_flat[s],
                "ids": prep_ids(ids_flat[s]),
                "wtbx": wtbx,
                "iota": _IOTA,
            }
        )
    return in_maps


_SHARDED_CACHE: dict = {}


def _get_sharded(nc):
    """Build (once) and cache the jitted SPMD executable for `nc`.

    Mirrors the multi-core branch of bass2jax.run_bass_via_pjrt, but keeps
    the jitted function across kernel() invocations — the stock path builds
    a fresh closure per call, which forces a full jax re-trace/compile each
    time (~7-9 s of repeat-call wall time).
    """
    if "entry" in _SHARDED_CACHE:
        return _SHARDED_CACHE["entry"]

    import jax
    from jax.experimental.shard_map import shard_map
    from jax.sharding import Mesh, PartitionSpec

    from concourse import bass2jax, mybir as _mybir

    bass2jax.install_neuronx_cc_hook()
    assert nc.dbg_addr is None
    partition_name = nc.partition_id_tensor.name if nc.partition_id_tensor else None

    in_names, out_names, out_avals = [], [], []
    for alloc in nc.m.functions[0].allocations:
        if not isinstance(alloc, _mybir.MemoryLocationSet):
            continue
        name = alloc.memorylocations[0].name
        if alloc.kind == "ExternalInput":
            if name != partition_name:
                in_names.append(name)
        elif alloc.kind == "ExternalOutput":
            shape = tuple(alloc.tensor_shape)
            out_avals.append(jax.core.ShapedArray(shape, _mybir.dt.np(alloc.dtype)))
            out_names.append(name)
    n_params, n_outs = len(in_names), len(out_names)
    all_names = in_names + out_names
    if partition_name is not None:
        all_names = all_names + [partition_name]
    donate = tuple(range(n_params, n_params + n_outs))

    def _body(*args):
        operands = list(args)
        if partition_name is not None:
            operands.append(bass2jax.partition_id_tensor())
        outs = bass2jax._bass_exec_p.bind(
            *operands,
            out_avals=tuple(out_avals),
            in_names=tuple(all_names),
            out_names=tuple(out_names),
            lowering_input_output_aliases=(),
            sim_require_finite=True,
            sim_require_nnan=True,
            nc=nc,
        )
        return tuple(outs)

    devices = jax.devices()[:N_CORES]
    mesh = Mesh(np.asarray(devices), ("core",))
    in_specs = (PartitionSpec("core"),) * (n_params + n_outs)
    out_specs = (PartitionSpec("core"),) * n_outs
    sharded = jax.jit(
        shard_map(
            _body, mesh=mesh, in_specs=in_specs, out_specs=out_specs, check_rep=False
        ),
        donate_argnums=donate,
        keep_unused=True,
    )
    entry = (sharded, in_names, out_names, out_avals)
    _SHARDED_CACHE["entry"] = entry
    return entry


def make_concat_inputs(lc, ids, W, b):
    """Globally concatenated (axis 0) per-core inputs for the cached SPMD
    path — avoids the per-core slice -> re-concat round-trip copies."""
    lc_flat = np.ascontiguousarray(np.asarray(lc, dtype=np.float32)).reshape(B * T, I)
    ids_flat = np.asarray(ids).reshape(B * T)
    ids_all = np.concatenate(
        [
            prep_ids(ids_flat[c * TOK_PER_CORE : (c + 1) * TOK_PER_CORE])
            for c in range(N_CORES)
        ]
    )
    wtbx = prep_wtbx(W, b)
    return {
        "lc": lc_flat,
        "ids": ids_all,
        "wtbx": np.tile(wtbx, (N_CORES, 1)),
        "iota": np.tile(_IOTA, (N_CORES, 1)),
    }


def _run_spmd_cached(nc, concat_inputs):
    """Returns the full concatenated output (B*T, 128)."""
    sharded, in_names, out_names, out_avals = _get_sharded(nc)
    concat_in = [concat_inputs[name] for name in in_names]
    concat_zeros = [
        np.zeros((N_CORES * a.shape[0], *a.shape[1:]), a.dtype) for a in out_avals
    ]
    out_arrs = sharded(*concat_in, *concat_zeros)
    i = out_names.index("out")
    return np.asarray(out_arrs[i]).reshape(B * T, I + N_EMBED)


def run(lc, ids, W, b, trace: bool = False):
    """Run on 8 NeuronCores; returns (full_output, BassKernelResults)."""
    nc = _get_nc()
    res = None
    try:
        out_flat = _run_spmd_cached(nc, make_concat_inputs(lc, ids, W, b))
    except Exception as e:  # noqa: BLE001 — fall back to the stock path
        print(f"kernel: cached SPMD path failed ({e!r}); using run_bass_kernel_spmd")
        in_maps = make_in_maps(lc, ids, W, b)
        res = run_bass_kernel_spmd(nc, in_maps, list(range(N_CORES)), trace=trace)
        out_flat = np.concatenate(
            [res.results[c]["out"] for c in range(N_CORES)], axis=0
        )
    out = out_flat.reshape(B, T, I + N_EMBED)
    return np.ascontiguousarray(out, dtype=np.float32), res


def kernel(lc, ids, W, b):
    out, _ = run(lc, ids, W, b)
    return out


if __name__ == "__main__":
    rng = np.random.default_rng(0)
    lc = rng.standard_normal((B, T, I), dtype=np.float32)
    ids = rng.integers(0, N_SPK, size=(B, T), dtype=np.int64)
    W = rng.standard_normal((N_EMBED, N_SPK), dtype=np.float32)
    b = rng.standard_normal((N_EMBED,), dtype=np.float32)
    out = kernel(lc=lc, ids=ids, W=W, b=b)
    exp = np.concatenate((lc, W.T[ids] + b), axis=2)
    err = np.max(np.abs(out - exp)) / np.max(np.abs(exp))
    print("max abs rel-to-scale err:", err)


# revision 10
# speedup vs baseline: 1.0553x; 1.0004x over previous
"""Trainium2 Bass kernel for nn_Conditioning (embedding lookup + concat).

Reference computation:
    gc = W.T[ids] + b          # (B, T, 64) gather from a tiny 128x64 table
    out = concat(lc, gc, -1)   # (B, T, 128)

Shapes: lc (16, 32768, 64) f32, ids (16, 32768) int64, W (64, 128) f32,
b (64,) f32 -> out (16, 32768, 128) f32.

Sharding: data-parallel over batch — 2 batches (65536 tokens) per core on
8 cores; the packed table and iota column replicated.

Device algorithm (per core), memory-roofline oriented (~50.5 MB DMA traffic
= ~140.3 us at 360 GB/s/core — loads + stores serialize on the DMA bus):
  * Host precomputes the packed bf16 gather table
    wtbx = [bf16(W.T + b) | bf16((W.T + b) - f32(bf16(W.T + b)))]  (hi|lo)
    and a (128, 1) f32 iota column, so the device does no constant setup.
  * ids are host-encoded to bf16 and slot-permuted per macro-tile.
  * Per macro-tile of 128*q tokens (token t = q*p + s <-> partition p,
    slot s; q=32 steady state with a short ramp-up):
      - ids row (1, 128q) bf16 DMA (Act HWDGE queue); GpSimd
        partition_broadcast of the row VIEWED AS int32 pairs (halves the
        modeled Q7 time, byte-identical result) -> (128, 128q) bf16.
      - DVE is_equal(ids_bcast, iota) -> one-hot (speaker, token) bf16,
        all-SBUF 2-byte operands (4x DVE mode).
      - Per 8-slot chunk: 8 matmul PAIRS accumulate hi+lo halves of the
        table into one (128, 8, 64) f32 PSUM bank (exact f32 re-add done
        by PE accumulation, same modeled PE cost as one 128-wide matmul);
        then a single copy PSUM -> out-tile gc columns (Act/DVE alternate).
      - lc DMA into staging (Act queue); Act copies it into the out tile's
        lc columns one half-macro at a time.
      - TWO contiguous stores per macro (half-macro each, SP queue) so
        stores start earlier and the final tail is short. Loads and stores
        live on different HWDGE queues so a store's semaphore wait never
        blocks the next load's issue; loads run 3 macros ahead.
"""

import sys

for _p in ("/opt/trn_rl_repo",):
    if _p not in sys.path:
        sys.path.insert(0, _p)

from contextlib import ExitStack

import ml_dtypes
import numpy as np

import concourse.bass as bass  # noqa: F401
import concourse.tile as tile
from concourse import bacc, mybir
from concourse.bass_utils import run_bass_kernel_spmd

N_CORES = 8
B, T, I = 16, 32768, 64
N_SPK, N_EMBED = 128, 64
P = 128  # partitions
TOK_PER_CORE = B * T // N_CORES  # 65536
# (tokens-per-partition q, macro count): short ramp-up then 4096-token macros
SCHEDULE = ((32, 16),)
CHUNK = 8  # psum tile = (128, CHUNK, 64) f32 = one 2 KB PSUM bank
LOOKAHEAD = 3  # macros of load lookahead on the Act queue

F32 = mybir.dt.float32
BF16 = mybir.dt.bfloat16
I32 = mybir.dt.int32

assert sum(P * q * c for q, c in SCHEDULE) == TOK_PER_CORE


def _macro_list(schedule):
    tok0, out = 0, []
    for q, cnt in schedule:
        for _ in range(cnt):
            out.append((tok0, q))
            tok0 += P * q
    return out, tok0


def build_bass(schedule=SCHEDULE):
    macros, tok = _macro_list(schedule)
    n_macros = len(macros)

    nc = bacc.Bacc("TRN2", target_bir_lowering=False, debug=False)
    lc = nc.dram_tensor("lc", (tok, I), F32, kind="ExternalInput").ap()
    ids = nc.dram_tensor("ids", (tok,), BF16, kind="ExternalInput").ap()
    wtbx_d = nc.dram_tensor("wtbx", (N_SPK, 2 * N_EMBED), BF16, kind="ExternalInput").ap()
    out = nc.dram_tensor("out", (tok, I + N_EMBED), F32, kind="ExternalOutput").ap()

    with tile.TileContext(nc) as tc, ExitStack() as ctx:
        const = ctx.enter_context(tc.tile_pool(name="const", bufs=1))
        ids_pool = ctx.enter_context(tc.tile_pool(name="idsrow", bufs=LOOKAHEAD + 2))
        bc_pool = ctx.enter_context(tc.tile_pool(name="idsbc", bufs=2))
        oh_pool = ctx.enter_context(tc.tile_pool(name="onehot", bufs=2))
        lc_pool = ctx.enter_context(tc.tile_pool(name="lct", bufs=LOOKAHEAD + 2))
        out_pool = ctx.enter_context(tc.tile_pool(name="outt", bufs=3))
        pgc_pool = ctx.enter_context(tc.tile_pool(name="pgc", bufs=6, space="PSUM"))

        # ---- load issue (Act queue), LOOKAHEAD macros ahead of compute ----
        t_ids, t_lc = {}, {}

        def load(k):
            tok0, q = macros[k]
            macro = P * q
            t_ids[k] = ids_pool.tile([1, macro], BF16, tag="ids_row", name=f"ids_row_{k}")
            nc.scalar.dma_start(
                out=t_ids[k][:], in_=ids[tok0 : tok0 + macro].rearrange("(o m) -> o m", o=1)
            )
            t_lc[k] = lc_pool.tile([P, q * I], F32, tag="lc_t", name=f"lc_t_{k}")
            # macro 0's lc rides the (otherwise idle) SP queue: it becomes the
            # very first HWDGE issue, so the big first transfer starts ~1.4 us
            # in while ids0 issues in parallel on Act
            eng = nc.sync if k == 0 else nc.scalar
            eng.dma_start(
                out=t_lc[k][:],
                in_=lc[tok0 : tok0 + macro, :].rearrange("(p q) d -> p (q d)", p=P, q=q),
            )

        def compute(k):
            tok0, q = macros[k]
            macro = P * q
            ids_row = t_ids.pop(k)
            lc_t = t_lc.pop(k)

            ids_bc = bc_pool.tile([P, macro], BF16, tag="ids_bc")
            nc.gpsimd.partition_broadcast(
                ids_bc[:].bitcast(I32), ids_row[:].bitcast(I32)
            )
            onehot = oh_pool.tile([P, macro], BF16, tag="onehot")
            nc.vector.tensor_scalar(
                out=onehot[:],
                in0=ids_bc[:],
                scalar1=iota_f[:],
                scalar2=None,
                op0=mybir.AluOpType.is_equal,
            )

            out_t = out_pool.tile([P, q, I + N_EMBED], F32, tag="out_t")
            out_re = out[tok0 : tok0 + macro, :].rearrange(
                "(p q) d -> p q d", p=P, q=q
            )
            chunk = min(CHUNK, q)
            n_chunks = q // chunk
            halves = 2 if n_chunks >= 2 else 1
            per_half = n_chunks // halves
            q2 = q // halves
            for s in range(halves):
                for hh in range(per_half):
                    h = s * per_half + hh
                    sl = slice(h * chunk, (h + 1) * chunk)
                    pg = pgc_pool.tile([P, chunk, N_EMBED], F32, tag="pg")
                    for jj in range(chunk):
                        j = h * chunk + jj
                        nc.tensor.matmul(
                            pg[:, jj, :],
                            lhsT=onehot[:, j * P : (j + 1) * P],
                            rhs=wtbx[:, 0:N_EMBED],
                            start=True,
                            stop=False,
                        )
                        nc.tensor.matmul(
                            pg[:, jj, :],
                            lhsT=onehot[:, j * P : (j + 1) * P],
                            rhs=wtbx[:, N_EMBED : 2 * N_EMBED],
                            start=False,
                            stop=True,
                        )
                    # single exact copy: PSUM already holds hi+lo re-added
                    if h % 2 == 0:
                        nc.scalar.copy(out_t[:, sl, I : I + N_EMBED], pg[:])
                    else:
                        nc.vector.tensor_copy(
                            out=out_t[:, sl, I : I + N_EMBED], in_=pg[:]
                        )
                shalf = slice(s * q2, (s + 1) * q2)
                nc.scalar.copy(
                    out_t[:, shalf, 0:I], lc_t[:, s * q2 * I : (s + 1) * q2 * I]
                )
                nc.sync.dma_start(out=out_re[:, shalf, :], in_=out_t[:, shalf, :])

        # first macro's loads are the very first HWDGE issues (ids0 then lc0);
        # the table constant rides the Pool SWDGE path so it never takes an
        # HWDGE slot in the critical head window; iota is built on-chip
        load(0)
        wtbx = const.tile([N_SPK, 2 * N_EMBED], BF16)
        nc.gpsimd.dma_start(out=wtbx[:], in_=wtbx_d[:])
        iota_i = const.tile([P, 1], mybir.dt.int32)
        nc.gpsimd.iota(iota_i[:], pattern=[[0, 1]], base=0, channel_multiplier=1)
        iota_f = const.tile([P, 1], F32)
        nc.vector.tensor_copy(out=iota_f[:], in_=iota_i[:])
        for k in range(1, min(LOOKAHEAD, n_macros)):
            load(k)
        for k in range(n_macros):
            if k + LOOKAHEAD < n_macros:
                load(k + LOOKAHEAD)
            compute(k)

    nc.compile()
    return nc


_NC_CACHE: dict = {}


def _get_nc(schedule=SCHEDULE):
    if schedule not in _NC_CACHE:
        _NC_CACHE[schedule] = build_bass(schedule)
    return _NC_CACHE[schedule]


def prep_ids(ids_shard_flat, schedule=SCHEDULE):
    """bf16-encode and slot-group a per-core flat ids shard.

    Within each macro of 128*q tokens, token t = q*p + s must appear at
    column s*128 + p so that matmul group s's one-hot columns line up with
    PSUM slot s (pure layout permutation; values unchanged).
    """
    a = np.asarray(ids_shard_flat).astype(np.float32).astype(ml_dtypes.bfloat16)
    macros, tok = _macro_list(schedule)
    assert a.shape == (tok,)
    parts = []
    for tok0, q in macros:
        parts.append(a[tok0 : tok0 + P * q].reshape(P, q).T.reshape(-1))
    return np.ascontiguousarray(np.concatenate(parts))


def prep_wtbx(W, b):
    """Packed bf16 hi|lo gather table: rows = speaker, cols = [hi64 | lo64]."""
    wtb = np.asarray(W, dtype=np.float32).T + np.asarray(b, dtype=np.float32)[None, :]
    hi = wtb.astype(ml_dtypes.bfloat16)
    lo = (wtb - hi.astype(np.float32)).astype(ml_dtypes.bfloat16)
    return np.ascontiguousarray(np.concatenate([hi, lo], axis=1))


def make_in_maps(lc, ids, W, b):
    """Shard full inputs into per-core input maps for the bass kernel."""
    lc_flat = np.ascontiguousarray(np.asarray(lc, dtype=np.float32)).reshape(B * T, I)
    ids_flat = np.asarray(ids).reshape(B * T)
    wtbx = prep_wtbx(W, b)
    in_maps = []
    for c in range(N_CORES):
        s = slice(c * TOK_PER_CORE, (c + 1) * TOK_PER_CORE)
        in_maps.append(
            {
                "lc": lc_flat[s],
                "ids": prep_ids(ids_flat[s]),
                "wtbx": wtbx,
            }
        )
    return in_maps


_SHARDED_CACHE: dict = {}


def _get_sharded(nc):
    """Build (once) and cache the jitted SPMD executable for `nc`.

    Mirrors the multi-core branch of bass2jax.run_bass_via_pjrt, but keeps
    the jitted function across kernel() invocations — the stock path builds
    a fresh closure per call, which forces a full jax re-trace/compile each
    time (~7-9 s of repeat-call wall time).
    """
    if "entry" in _SHARDED_CACHE:
        return _SHARDED_CACHE["entry"]

    import jax
    from jax.experimental.shard_map import shard_map
    from jax.sharding import Mesh, PartitionSpec

    from concourse import bass2jax, mybir as _mybir

    bass2jax.install_neuronx_cc_hook()
    assert nc.dbg_addr is None
    partition_name = nc.partition_id_tensor.name if nc.partition_id_tensor else None

    in_names, out_names, out_avals = [], [], []
    for alloc in nc.m.functions[0].allocations:
        if not isinstance(alloc, _mybir.MemoryLocationSet):
            continue
        name = alloc.memorylocations[0].name
        if alloc.kind == "ExternalInput":
            if name != partition_name:
                in_names.append(name)
        elif alloc.kind == "ExternalOutput":
            shape = tuple(alloc.tensor_shape)
            out_avals.append(jax.core.ShapedArray(shape, _mybir.dt.np(alloc.dtype)))
            out_names.append(name)
    n_params, n_outs = len(in_names), len(out_names)
    all_names = in_names + out_names
    if partition_name is not None:
        all_names = all_names + [partition_name]
    donate = tuple(range(n_params, n_params + n_outs))

    def _body(*args):
        operands = list(args)
        if partition_name is not None:
            operands.append(bass2jax.partition_id_tensor())
        outs = bass2jax._bass_exec_p.bind(
            *operands,
            out_avals=tuple(out_avals),
            in_names=tuple(all_names),
            out_names=tuple(out_names),
            lowering_input_output_aliases=(),
            sim_require_finite=True,
            sim_require_nnan=True,
            nc=nc,
        )
        return tuple(outs)

    devices = jax.devices()[:N_CORES]
    mesh = Mesh(np.asarray(devices), ("core",))
    in_specs = (PartitionSpec("core"),) * (n_params + n_outs)
    out_specs = (PartitionSpec("core"),) * n_outs
    sharded = jax.jit(
        shard_map(
            _body, mesh=mesh, in_specs=in_specs, out_specs=out_specs, check_rep=False
        ),
        donate_argnums=donate,
        keep_unused=True,
    )
    entry = (sharded, in_names, out_names, out_avals)
    _SHARDED_CACHE["entry"] = entry
    return entry


def make_concat_inputs(lc, ids, W, b):
    """Globally concatenated (axis 0) per-core inputs for the cached SPMD
    path — avoids the per-core slice -> re-concat round-trip copies."""
    lc_flat = np.ascontiguousarray(np.asarray(lc, dtype=np.float32)).reshape(B * T, I)
    ids_flat = np.asarray(ids).reshape(B * T)
    ids_all = np.concatenate(
        [
            prep_ids(ids_flat[c * TOK_PER_CORE : (c + 1) * TOK_PER_CORE])
            for c in range(N_CORES)
        ]
    )
    wtbx = prep_wtbx(W, b)
    return {
        "lc": lc_flat,
        "ids": ids_all,
        "wtbx": np.tile(wtbx, (N_CORES, 1)),
    }


def _run_spmd_cached(nc, concat_inputs):
    """Returns the full concatenated output (B*T, 128)."""
    sharded, in_names, out_names, out_avals = _get_sharded(nc)
    concat_in = [concat_inputs[name] for name in in_names]
    concat_zeros = [
        np.zeros((N_CORES * a.shape[0], *a.shape[1:]), a.dtype) for a in out_avals
    ]
    out_arrs = sharded(*concat_in, *concat_zeros)
    i = out_names.index("out")
    return np.asarray(out_arrs[i]).reshape(B * T, I + N_EMBED)


def run(lc, ids, W, b, trace: bool = False):
    """Run on 8 NeuronCores; returns (full_output, BassKernelResults)."""
    nc = _get_nc()
    res = None
    try:
        out_flat = _run_spmd_cached(nc, make_concat_inputs(lc, ids, W, b))
    except Exception as e:  # noqa: BLE001 — fall back to the stock path
        print(f"kernel: cached SPMD path failed ({e!r}); using run_bass_kernel_spmd")
        in_maps = make_in_maps(lc, ids, W, b)
        res = run_bass_kernel_spmd(nc, in_maps, list(range(N_CORES)), trace=trace)
        out_flat = np.concatenate(
            [res.results[c]["out"] for c in range(N_CORES)], axis=0
        )
    out = out_flat.reshape(B, T, I + N_EMBED)
    return np.ascontiguousarray(out, dtype=np.float32), res


def kernel(lc, ids, W, b):
    out, _ = run(lc, ids, W, b)
    return out


if __name__ == "__main__":
    rng = np.random.default_rng(0)
    lc = rng.standard_normal((B, T, I), dtype=np.float32)
    ids = rng.integers(0, N_SPK, size=(B, T), dtype=np.int64)
    W = rng.standard_normal((N_EMBED, N_SPK), dtype=np.float32)
    b = rng.standard_normal((N_EMBED,), dtype=np.float32)
    out = kernel(lc=lc, ids=ids, W=W, b=b)
    exp = np.concatenate((lc, W.T[ids] + b), axis=2)
    err = np.max(np.abs(out - exp)) / np.max(np.abs(exp))
    print("max abs rel-to-scale err:", err)
